# revision 28
# baseline (speedup 1.0000x reference)
"""Trainium2 Bass kernel for batched unscaled dot-product attention with
input projections (torch-Linear style):

    q = query @ Wq.T + bq ; k = keys @ Wk.T + bk ; v = values @ Wv.T + bv
    out = softmax(q @ k.T, axis=-1) @ v

Shapes: query/keys/values [B=8, S=4096, D=256]; W* [256, 256]; b* [256].

Strategy (data-parallel over batch, one batch element per NeuronCore):

Algebraic restructuring so NO tensor ever needs an HBM-side transpose and
the K/V projections fold away:
  - softmax(q@k.T) rows are invariant to adding per-row constants, so with
    A = query @ (Wq.T @ Wk) + 1*(bq @ Wk)   [4096, 256]
    softmax(q @ k.T) == softmax(A @ keys.T)   (bk drops out entirely).
  - out = P @ v = (P @ values) @ Wv.T + 1*bv  (P rows sum to 1), so the
    V projection is applied AFTER the attention-weighted sum.

On-chip pipeline per core (S^T layout — keys on PSUM partitions):
  1. prologue: ~3.4us of dummy matmuls open the PE clock-gate (HAM);
     Mw = Wq.T@Wk, c = Wk.T@bq on PE (tiny); PE-transpose query tiles;
     A^T = Mw^T q^T + c. keys-transposes and values-rounding are folded
     into the first i-tile's main loop for PE density.
  2. main loop over (i-tile of 512 query cols) x (j-chunk of 128 keys):
       S^T[j, i]  = kT.T @ A^T   (2 fp32r matmuls, PSUM)
       P^T        = exp(S^T)     (ScalarE, PSUM->SBUF; |scores| <~ 40 so
                                  exp() needs no max-subtraction in fp32)
       ctx^T     += values^T @ P^T  (2 fp32r matmuls, PSUM accum over j)
       acc       += P^T             (VectorE running sum for denominators)
     The NEXT i-tile's query transposes + A-projection are emitted
     mid-loop (jt==16) so their copies don't collide with the exp drain
     at the i-tile boundary.
  3. per i-tile: den[i] = acc.T @ ones — 4 tiny matmuls that land the
     denominators directly in [query-on-partition] column layout;
     recip on VectorE.
  4. out[i, d] = (ctx^T.T @ Wv^T) * recip + bv  (2 matmuls + DVE per
     128-row chunk), emitted one i-tile behind the main loop.

All big matmuls use float32r (full PE speed at free-dim>=256, ~1.5e-4
relative error vs 2.3e-3 for bf16 — measured on HW).
"""

import numpy as np

import concourse.bass as bass
import concourse.tile as tile
from concourse import bacc, mybir
from concourse.bass_utils import run_bass_kernel_spmd
from concourse.masks import make_identity

P = 128
D = 256
DC = D // P  # 2 chunks of the feature dim
IT = 512     # i-tile (query positions per main-loop tile)
ICPT = IT // P  # output row chunks per i-tile
N_CORES = 8
PIPE_DEPTH = 2  # ctx-matmul emission lag behind exp, in j-chunks

F32 = mybir.dt.float32
F32R = mybir.dt.float32r
EXP = mybir.ActivationFunctionType.Exp


def build_attention(S_q: int, S_k: int, num_devices: int = N_CORES):
    assert S_q % IT == 0 and S_k % P == 0
    NI = S_q // IT   # i-tiles
    NJ = S_k // P    # j-chunks
    NIC = S_q // P   # output row chunks

    nc = bacc.Bacc(
        "TRN2",
        target_bir_lowering=False,
        debug=False,
        enable_asserts=False,
        num_devices=num_devices,
    )

    q_d = nc.dram_tensor("query", [S_q, D], F32, kind="ExternalInput").ap()
    k_d = nc.dram_tensor("keys", [S_k, D], F32, kind="ExternalInput").ap()
    v_d = nc.dram_tensor("values", [S_k, D], F32, kind="ExternalInput").ap()
    wq_d = nc.dram_tensor("Wq", [D, D], F32, kind="ExternalInput").ap()
    wk_d = nc.dram_tensor("Wk", [D, D], F32, kind="ExternalInput").ap()
    wv_d = nc.dram_tensor("Wv", [D, D], F32, kind="ExternalInput").ap()
    bq_d = nc.dram_tensor("bq", [D], F32, kind="ExternalInput").ap()
    bv_d = nc.dram_tensor("bv", [D], F32, kind="ExternalInput").ap()
    out_d = nc.dram_tensor("out", [S_q, D], F32, kind="ExternalOutput").ap()

    with tile.TileContext(nc) as tc:
        with (
            tc.tile_pool(name="persist", bufs=1) as persist,
            tc.tile_pool(name="pre_in", bufs=6) as kin,
            tc.tile_pool(name="qts", bufs=2) as qts_pool,
            tc.tile_pool(name="s_ps", bufs=4, space="PSUM") as s_pool,
            tc.tile_pool(name="acc_ps", bufs=1, space="PSUM") as acc_pool,
            tc.tile_pool(name="o_ps", bufs=2, space="PSUM") as o_pool,
            tc.tile_pool(name="p_sb", bufs=PIPE_DEPTH + 2) as p_pool,
            tc.tile_pool(name="acc_sb", bufs=2) as accs_pool,
            tc.tile_pool(name="fin", bufs=3) as fin,
        ):
            kT = persist.tile([P, DC, S_k], F32R, tag="kT")       # keys^T
            aT = persist.tile([P, DC, S_q], F32R, tag="aT")       # A^T
            vr = persist.tile([P, NJ, D], F32R, tag="vr")         # values (rounded)
            ctxT = persist.tile([P, DC, S_q], F32R, tag="ctxT")   # (P@values)^T
            mw = persist.tile([P, DC, D], F32R, tag="mw")         # Wq.T@Wk
            wq = persist.tile([P, DC, D], F32, tag="wq")
            wk = persist.tile([P, DC, D], F32, tag="wk")
            wv = persist.tile([P, DC, D], F32, tag="wv")
            wvT = persist.tile([P, DC, D], F32R, tag="wvT")       # Wv^T
            cvec = persist.tile([P, DC], F32, tag="cvec")         # Wk.T@bq
            bqc = persist.tile([P, DC], F32, tag="bqc")
            wq_r = persist.tile([P, DC, D], F32R, tag="wq_r")
            wk_r = persist.tile([P, DC, D], F32R, tag="wk_r")
            bqc_r = persist.tile([P, DC], F32R, tag="bqc_r")
            ones_f = persist.tile([P, 1], F32, tag="ones_f")
            ones = persist.tile([P, 1], F32R, tag="ones")
            ident = persist.tile([P, P], F32, tag="ident")
            bvb = persist.tile([P, D], F32, tag="bvb")            # bv bcast
            denpf = persist.tile([P, NI, ICPT], F32, tag="denpf")
            recip = persist.tile([P, NI, ICPT], F32, tag="recip")
            wtile = persist.tile([P, IT], F32, tag="warm")

            nc.vector.memset(ones_f[:], 1.0)
            nc.vector.tensor_copy(ones[:], ones_f[:])
            make_identity(nc, ident[:])

            # HAM warmup: ~3.4us of real matmul activity un-throttles the PE
            # clock (1.2 -> 2.4 GHz) before the real pipeline begins.
            nc.vector.memset(wtile[:], 0.0)
            for _ in range(2):
                wps = s_pool.tile([P, IT], F32, tag="s")
                nc.tensor.matmul(wps[:], wtile[:, :P], wtile[:], start=True, stop=True)

            def emit_keepalive(n=P):
                # transposes don't register as PE-busy in the clock-gate's
                # activity window; a short real matmul does.
                wps = s_pool.tile([P, IT], F32, tag="s")
                nc.tensor.matmul(wps[:, :n], wtile[:, :P], wtile[:, :n], start=True, stop=True)

            def emit_weight_dmas():
                nc.gpsimd.dma_start(wq[:], wq_d.rearrange("(no ni) d -> ni no d", ni=P))
                nc.gpsimd.dma_start(wk[:], wk_d.rearrange("(no ni) d -> ni no d", ni=P))
                nc.gpsimd.dma_start(wv[:], wv_d.rearrange("(do p) di -> p do di", p=P))
                nc.gpsimd.dma_start(bqc[:], bq_d.rearrange("(no ni) -> ni no", ni=P))
                nc.gpsimd.dma_start(bvb[:], bv_d.unsqueeze(0).to_broadcast([P, D]))

            def emit_weight_prep():
                for dc_ in range(DC):
                    nc.any.tensor_copy(wq_r[:, dc_, :], wq[:, dc_, :])
                    nc.any.tensor_copy(wk_r[:, dc_, :], wk[:, dc_, :])
                nc.any.tensor_copy(bqc_r[:], bqc[:])
                for dic in range(DC):
                    mps = s_pool.tile([P, D], F32, tag="s")
                    for no in range(DC):
                        nc.tensor.matmul(
                            mps[:, :D],
                            wq_r[:, no, dic * P:(dic + 1) * P], wk_r[:, no, :],
                            start=(no == 0), stop=(no == DC - 1),
                        )
                    nc.any.tensor_copy(mw[:, dic, :], mps[:, :D])
                for kc in range(DC):
                    cps = s_pool.tile([P, 1], F32, tag="s")
                    for no in range(DC):
                        nc.tensor.matmul(
                            cps[:], wk[:, no, kc * P:(kc + 1) * P], bqc[:, no:no + 1],
                            start=(no == 0), stop=(no == DC - 1),
                        )
                    nc.any.tensor_copy(cvec[:, kc:kc + 1], cps[:])
                for a_ in range(DC):
                    for b_ in range(DC):
                        tps = s_pool.tile([P, P], F32, tag="s")
                        nc.tensor.transpose(tps[:, :P], wv[:, a_, b_ * P:(b_ + 1) * P], ident[:])
                        nc.any.tensor_copy(wvT[:, b_, a_ * P:(a_ + 1) * P], tps[:, :P])

            # ---- helpers ----
            def emit_ktr(jt):
                """DMA a 128-row chunk of keys, PE-transpose to kT."""
                ktile = kin.tile([P, D], F32, tag="ktile")
                nc.sync.dma_start(ktile[:], k_d[jt * P:(jt + 1) * P, :])
                for dc_ in range(DC):
                    tp = s_pool.tile([P, P], F32, tag="s")
                    nc.tensor.transpose(tp[:, :P], ktile[:, dc_ * P:(dc_ + 1) * P], ident[:])
                    nc.any.tensor_copy(kT[:, dc_, jt * P:(jt + 1) * P], tp[:, :P])

            def emit_vload(jt):
                vtile = kin.tile([P, D], F32, tag="vtile")
                nc.sync.dma_start(vtile[:], v_d[jt * P:(jt + 1) * P, :])
                nc.gpsimd.tensor_copy(vr[:, jt, :], vtile[:])

            def emit_q_tr(it):
                """Transpose 4 query chunks into a qTs staging tile."""
                qTs = qts_pool.tile([P, DC, IT], F32R, tag="qTs")
                for ii in range(ICPT):
                    r0 = it * IT + ii * P
                    qtile = kin.tile([P, D], F32, tag="qtile")
                    nc.sync.dma_start(qtile[:], q_d[r0:r0 + P, :])
                    for dc_ in range(DC):
                        tp = s_pool.tile([P, P], F32, tag="s")
                        nc.tensor.transpose(tp[:, :P], qtile[:, dc_ * P:(dc_ + 1) * P], ident[:])
                        nc.any.tensor_copy(qTs[:, dc_, ii * P:(ii + 1) * P], tp[:, :P])
                return qTs

            def emit_a_proj(it, qTs):
                for kc in range(DC):
                    aps = s_pool.tile([P, IT], F32, tag="s")
                    for dc_ in range(DC):
                        nc.tensor.matmul(
                            aps[:], mw[:, dc_, kc * P:(kc + 1) * P], qTs[:, dc_, :],
                            start=(dc_ == 0), stop=(dc_ == DC - 1),
                        )
                    nc.any.tensor_scalar_add(
                        aT[:, kc, it * IT:(it + 1) * IT], aps[:], cvec[:, kc:kc + 1]
                    )

            def emit_q_block(it):
                emit_a_proj(it, emit_q_tr(it))

            def emit_final(it):
                """recip + output projection + bias + store for one i-tile."""
                nc.vector.reciprocal(recip[:, it, :], denpf[:, it, :])
                for ii in range(ICPT):
                    ic = it * ICPT + ii
                    o_ps = o_pool.tile([P, D], F32, tag="o")
                    for dc_ in range(DC):
                        nc.tensor.matmul(
                            o_ps[:], ctxT[:, dc_, ic * P:(ic + 1) * P], wvT[:, dc_, :],
                            start=(dc_ == 0), stop=(dc_ == DC - 1),
                        )
                    t1 = fin.tile([P, D], F32, tag="t1")
                    nc.any.tensor_scalar_mul(t1[:], o_ps[:], recip[:, it, ii:ii + 1])
                    t2 = fin.tile([P, D], F32, tag="t2")
                    nc.vector.tensor_add(t2[:], t1[:], bvb[:])
                    nc.sync.dma_start(out_d[ic * P:(ic + 1) * P, :], t2[:])

            # ---- prologue ----
            KTR_LEAD, V_LEAD = 2, 4
            emit_weight_dmas()
            qTs0 = emit_q_tr(0)
            emit_keepalive()
            for jt in range(min(KTR_LEAD, NJ)):
                emit_ktr(jt)
                emit_keepalive()
            for jt in range(min(V_LEAD, NJ)):
                emit_vload(jt)
            emit_weight_prep()
            emit_a_proj(0, qTs0)

            # ---- main loop ----
            for it in range(NI):
                i0 = it * IT
                ctx_ps = [
                    acc_pool.tile([P, IT], F32, tag=f"ctx{dc_}", name=f"ctx_ps{dc_}")
                    for dc_ in range(DC)
                ]
                acc = accs_pool.tile([P, IT], F32R, tag="acc")

                def emit_ctx(psb, jt):
                    st, sp = (jt == 0), (jt == NJ - 1)
                    for dc_ in range(DC):
                        nc.tensor.matmul(
                            ctx_ps[dc_][:], vr[:, jt, dc_ * P:(dc_ + 1) * P], psb[:],
                            start=st, stop=sp,
                        )

                pending = []
                for jt in range(NJ):
                    if it == 0:
                        if jt + KTR_LEAD < NJ:
                            emit_ktr(jt + KTR_LEAD)
                        if jt + V_LEAD < NJ:
                            emit_vload(jt + V_LEAD)
                    if jt == (24 if it == 0 else 16) and it + 1 < NI:
                        emit_q_block(it + 1)
                    sps = s_pool.tile([P, IT], F32, tag="s")
                    for dc_ in range(DC):
                        nc.tensor.matmul(
                            sps[:], kT[:, dc_, jt * P:(jt + 1) * P], aT[:, dc_, i0:i0 + IT],
                            start=(dc_ == 0), stop=(dc_ == DC - 1),
                        )
                    psb = p_pool.tile([P, IT], F32R, tag="p")
                    nc.scalar.activation(psb[:], sps[:], EXP)
                    if jt == 0:
                        nc.vector.tensor_copy(acc[:], psb[:])
                    else:
                        nc.vector.tensor_add(acc[:], acc[:], psb[:])
                    pending.append((psb, jt))
                    if len(pending) > PIPE_DEPTH:
                        emit_ctx(*pending.pop(0))
                for args in pending:
                    emit_ctx(*args)

                # denominators, directly in column layout: den[i] = acc.T @ ones
                den_ps = o_pool.tile([P, ICPT], F32, tag="o")
                for ii in range(ICPT):
                    nc.tensor.matmul(
                        den_ps[:, ii:ii + 1],
                        acc[:, ii * P:(ii + 1) * P].bitcast(F32),
                        ones_f[:],
                        start=(ii == 0), stop=(ii == ICPT - 1),
                    )
                nc.any.tensor_copy(denpf[:, it, :], den_ps[:])

                if it >= 1:
                    emit_final(it - 1)
                for dc_ in range(DC):
                    nc.any.tensor_copy(ctxT[:, dc_, i0:i0 + IT], ctx_ps[dc_][:])
            emit_final(NI - 1)

    nc.compile()
    return nc


_CACHE: dict = {}


def _get_nc(S_q, S_k, num_devices):
    key = (S_q, S_k, num_devices)
    if key not in _CACHE:
        _CACHE[key] = build_attention(S_q, S_k, num_devices)
    return _CACHE[key]


def kernel(query, keys, values, Wq, bq, Wk, bk, Wv, bv, **_unused):
    """Full-input entry point: shards batch across 8 NeuronCores."""
    query = np.ascontiguousarray(query, dtype=np.float32)
    keys = np.ascontiguousarray(keys, dtype=np.float32)
    values = np.ascontiguousarray(values, dtype=np.float32)
    B, S_q, d = query.shape
    S_k = keys.shape[1]
    assert d == D and B == N_CORES

    nc = _get_nc(S_q, S_k, N_CORES)
    shared = {
        "Wq": np.ascontiguousarray(Wq, dtype=np.float32),
        "Wk": np.ascontiguousarray(Wk, dtype=np.float32),
        "Wv": np.ascontiguousarray(Wv, dtype=np.float32),
        "bq": np.ascontiguousarray(bq, dtype=np.float32),
        "bv": np.ascontiguousarray(bv, dtype=np.float32),
    }
    in_maps = [
        {"query": query[c], "keys": keys[c], "values": values[c], **shared}
        for c in range(B)
    ]
    res = run_bass_kernel_spmd(nc, in_maps, core_ids=list(range(N_CORES)))
    return np.stack([res.results[c]["out"] for c in range(B)], axis=0)


# revision 29
# speedup vs baseline: 1.0050x; 1.0050x over previous
"""Trainium2 Bass kernel for batched unscaled dot-product attention with
input projections (torch-Linear style):

    q = query @ Wq.T + bq ; k = keys @ Wk.T + bk ; v = values @ Wv.T + bv
    out = softmax(q @ k.T, axis=-1) @ v

Shapes: query/keys/values [B=8, S=4096, D=256]; W* [256, 256]; b* [256].

Strategy (data-parallel over batch, one batch element per NeuronCore):

Algebraic restructuring so NO tensor ever needs an HBM-side transpose and
the K/V projections fold away:
  - softmax(q@k.T) rows are invariant to adding per-row constants, so with
    A = query @ (Wq.T @ Wk) + 1*(bq @ Wk)   [4096, 256]
    softmax(q @ k.T) == softmax(A @ keys.T)   (bk drops out entirely).
  - out = P @ v = (P @ values) @ Wv.T + 1*bv  (P rows sum to 1), so the
    V projection is applied AFTER the attention-weighted sum.

On-chip pipeline per core (S^T layout — keys on PSUM partitions):
  1. prologue: ~3.4us of dummy matmuls open the PE clock-gate (HAM);
     Mw = Wq.T@Wk, c = Wk.T@bq on PE (tiny); PE-transpose query tiles;
     A^T = Mw^T q^T + c. keys-transposes and values-rounding are folded
     into the first i-tile's main loop for PE density.
  2. main loop over (i-tile of 512 query cols) x (j-chunk of 128 keys):
       S^T[j, i]  = kT.T @ A^T   (2 fp32r matmuls, PSUM)
       P^T        = exp(S^T)     (ScalarE, PSUM->SBUF; |scores| <~ 40 so
                                  exp() needs no max-subtraction in fp32)
       ctx^T     += values^T @ P^T  (2 fp32r matmuls, PSUM accum over j)
       acc       += P^T             (VectorE running sum for denominators)
     The NEXT i-tile's query transposes + A-projection are emitted
     mid-loop (jt==16) so their copies don't collide with the exp drain
     at the i-tile boundary.
  3. per i-tile: den[i] = acc.T @ ones — 4 tiny matmuls that land the
     denominators directly in [query-on-partition] column layout;
     recip on VectorE.
  4. out[i, d] = (ctx^T.T @ Wv^T) * recip + bv  (2 matmuls + DVE per
     128-row chunk), emitted one i-tile behind the main loop.

All big matmuls use float32r (full PE speed at free-dim>=256, ~1.5e-4
relative error vs 2.3e-3 for bf16 — measured on HW).
"""

import numpy as np

import concourse.bass as bass
import concourse.tile as tile
from concourse import bacc, mybir
from concourse.bass_utils import run_bass_kernel_spmd
from concourse.masks import make_identity

P = 128
D = 256
DC = D // P  # 2 chunks of the feature dim
IT = 512     # i-tile (query positions per main-loop tile)
ICPT = IT // P  # output row chunks per i-tile
N_CORES = 8
PIPE_DEPTH = 3  # ctx-matmul emission lag behind exp, in j-chunks

F32 = mybir.dt.float32
F32R = mybir.dt.float32r
EXP = mybir.ActivationFunctionType.Exp


def build_attention(S_q: int, S_k: int, num_devices: int = N_CORES):
    assert S_q % IT == 0 and S_k % P == 0
    NI = S_q // IT   # i-tiles
    NJ = S_k // P    # j-chunks
    NIC = S_q // P   # output row chunks

    nc = bacc.Bacc(
        "TRN2",
        target_bir_lowering=False,
        debug=False,
        enable_asserts=False,
        num_devices=num_devices,
    )

    q_d = nc.dram_tensor("query", [S_q, D], F32, kind="ExternalInput").ap()
    k_d = nc.dram_tensor("keys", [S_k, D], F32, kind="ExternalInput").ap()
    v_d = nc.dram_tensor("values", [S_k, D], F32, kind="ExternalInput").ap()
    wq_d = nc.dram_tensor("Wq", [D, D], F32, kind="ExternalInput").ap()
    wk_d = nc.dram_tensor("Wk", [D, D], F32, kind="ExternalInput").ap()
    wv_d = nc.dram_tensor("Wv", [D, D], F32, kind="ExternalInput").ap()
    bq_d = nc.dram_tensor("bq", [D], F32, kind="ExternalInput").ap()
    bv_d = nc.dram_tensor("bv", [D], F32, kind="ExternalInput").ap()
    out_d = nc.dram_tensor("out", [S_q, D], F32, kind="ExternalOutput").ap()

    with tile.TileContext(nc) as tc:
        with (
            tc.tile_pool(name="persist", bufs=1) as persist,
            tc.tile_pool(name="pre_in", bufs=6) as kin,
            tc.tile_pool(name="qts", bufs=2) as qts_pool,
            tc.tile_pool(name="s_ps", bufs=4, space="PSUM") as s_pool,
            tc.tile_pool(name="acc_ps", bufs=1, space="PSUM") as acc_pool,
            tc.tile_pool(name="o_ps", bufs=2, space="PSUM") as o_pool,
            tc.tile_pool(name="p_sb", bufs=PIPE_DEPTH + 2) as p_pool,
            tc.tile_pool(name="acc_sb", bufs=2) as accs_pool,
            tc.tile_pool(name="fin", bufs=3) as fin,
        ):
            kT = persist.tile([P, DC, S_k], F32R, tag="kT")       # keys^T
            aT = persist.tile([P, DC, S_q], F32R, tag="aT")       # A^T
            vr = persist.tile([P, NJ, D], F32R, tag="vr")         # values (rounded)
            ctxT = persist.tile([P, DC, S_q], F32R, tag="ctxT")   # (P@values)^T
            mw = persist.tile([P, DC, D], F32R, tag="mw")         # Wq.T@Wk
            wq = persist.tile([P, DC, D], F32, tag="wq")
            wk = persist.tile([P, DC, D], F32, tag="wk")
            wv = persist.tile([P, DC, D], F32, tag="wv")
            wvT = persist.tile([P, DC, D], F32R, tag="wvT")       # Wv^T
            cvec = persist.tile([P, DC], F32, tag="cvec")         # Wk.T@bq
            bqc = persist.tile([P, DC], F32, tag="bqc")
            wq_r = persist.tile([P, DC, D], F32R, tag="wq_r")
            wk_r = persist.tile([P, DC, D], F32R, tag="wk_r")
            bqc_r = persist.tile([P, DC], F32R, tag="bqc_r")
            ones_f = persist.tile([P, 1], F32, tag="ones_f")
            ones = persist.tile([P, 1], F32R, tag="ones")
            ident = persist.tile([P, P], F32, tag="ident")
            bvb = persist.tile([P, D], F32, tag="bvb")            # bv bcast
            denpf = persist.tile([P, NI, ICPT], F32, tag="denpf")
            recip = persist.tile([P, NI, ICPT], F32, tag="recip")
            wtile = persist.tile([P, IT], F32, tag="warm")

            nc.vector.memset(ones_f[:], 1.0)
            nc.vector.tensor_copy(ones[:], ones_f[:])
            make_identity(nc, ident[:])

            # HAM warmup: ~3.4us of real matmul activity un-throttles the PE
            # clock (1.2 -> 2.4 GHz) before the real pipeline begins.
            nc.vector.memset(wtile[:], 0.0)
            for _ in range(2):
                wps = s_pool.tile([P, IT], F32, tag="s")
                nc.tensor.matmul(wps[:], wtile[:, :P], wtile[:], start=True, stop=True)

            def emit_keepalive(n=P):
                # transposes don't register as PE-busy in the clock-gate's
                # activity window; a short real matmul does.
                wps = s_pool.tile([P, IT], F32, tag="s")
                nc.tensor.matmul(wps[:, :n], wtile[:, :P], wtile[:, :n], start=True, stop=True)

            def emit_weight_dmas():
                nc.gpsimd.dma_start(wq[:], wq_d.rearrange("(no ni) d -> ni no d", ni=P))
                nc.gpsimd.dma_start(wk[:], wk_d.rearrange("(no ni) d -> ni no d", ni=P))
                nc.gpsimd.dma_start(wv[:], wv_d.rearrange("(do p) di -> p do di", p=P))
                nc.gpsimd.dma_start(bqc[:], bq_d.rearrange("(no ni) -> ni no", ni=P))
                nc.gpsimd.dma_start(bvb[:], bv_d.unsqueeze(0).to_broadcast([P, D]))

            def emit_weight_prep():
                for dc_ in range(DC):
                    nc.any.tensor_copy(wq_r[:, dc_, :], wq[:, dc_, :])
                    nc.any.tensor_copy(wk_r[:, dc_, :], wk[:, dc_, :])
                nc.any.tensor_copy(bqc_r[:], bqc[:])
                for dic in range(DC):
                    mps = s_pool.tile([P, D], F32, tag="s")
                    for no in range(DC):
                        nc.tensor.matmul(
                            mps[:, :D],
                            wq_r[:, no, dic * P:(dic + 1) * P], wk_r[:, no, :],
                            start=(no == 0), stop=(no == DC - 1),
                        )
                    nc.any.tensor_copy(mw[:, dic, :], mps[:, :D])
                for kc in range(DC):
                    cps = s_pool.tile([P, 1], F32, tag="s")
                    for no in range(DC):
                        nc.tensor.matmul(
                            cps[:], wk[:, no, kc * P:(kc + 1) * P], bqc[:, no:no + 1],
                            start=(no == 0), stop=(no == DC - 1),
                        )
                    nc.any.tensor_copy(cvec[:, kc:kc + 1], cps[:])
                for a_ in range(DC):
                    for b_ in range(DC):
                        tps = s_pool.tile([P, P], F32, tag="s")
                        nc.tensor.transpose(tps[:, :P], wv[:, a_, b_ * P:(b_ + 1) * P], ident[:])
                        nc.any.tensor_copy(wvT[:, b_, a_ * P:(a_ + 1) * P], tps[:, :P])

            # ---- helpers ----
            def emit_ktr(jt):
                """DMA a 128-row chunk of keys, PE-transpose to kT."""
                ktile = kin.tile([P, D], F32, tag="ktile")
                nc.sync.dma_start(ktile[:], k_d[jt * P:(jt + 1) * P, :])
                for dc_ in range(DC):
                    tp = s_pool.tile([P, P], F32, tag="s")
                    nc.tensor.transpose(tp[:, :P], ktile[:, dc_ * P:(dc_ + 1) * P], ident[:])
                    nc.any.tensor_copy(kT[:, dc_, jt * P:(jt + 1) * P], tp[:, :P])

            def emit_vload(jt):
                vtile = kin.tile([P, D], F32, tag="vtile")
                nc.sync.dma_start(vtile[:], v_d[jt * P:(jt + 1) * P, :])
                nc.gpsimd.tensor_copy(vr[:, jt, :], vtile[:])

            def emit_q_tr(it):
                """Transpose 4 query chunks into a qTs staging tile."""
                qTs = qts_pool.tile([P, DC, IT], F32R, tag="qTs")
                for ii in range(ICPT):
                    r0 = it * IT + ii * P
                    qtile = kin.tile([P, D], F32, tag="qtile")
                    nc.sync.dma_start(qtile[:], q_d[r0:r0 + P, :])
                    for dc_ in range(DC):
                        tp = s_pool.tile([P, P], F32, tag="s")
                        nc.tensor.transpose(tp[:, :P], qtile[:, dc_ * P:(dc_ + 1) * P], ident[:])
                        nc.any.tensor_copy(qTs[:, dc_, ii * P:(ii + 1) * P], tp[:, :P])
                return qTs

            def emit_a_proj(it, qTs):
                for kc in range(DC):
                    aps = s_pool.tile([P, IT], F32, tag="s")
                    for dc_ in range(DC):
                        nc.tensor.matmul(
                            aps[:], mw[:, dc_, kc * P:(kc + 1) * P], qTs[:, dc_, :],
                            start=(dc_ == 0), stop=(dc_ == DC - 1),
                        )
                    nc.any.tensor_scalar_add(
                        aT[:, kc, it * IT:(it + 1) * IT], aps[:], cvec[:, kc:kc + 1]
                    )

            def emit_q_block(it):
                emit_a_proj(it, emit_q_tr(it))

            def emit_final(it):
                """recip + output projection + bias + store for one i-tile."""
                nc.vector.reciprocal(recip[:, it, :], denpf[:, it, :])
                for ii in range(ICPT):
                    ic = it * ICPT + ii
                    o_ps = o_pool.tile([P, D], F32, tag="o")
                    for dc_ in range(DC):
                        nc.tensor.matmul(
                            o_ps[:], ctxT[:, dc_, ic * P:(ic + 1) * P], wvT[:, dc_, :],
                            start=(dc_ == 0), stop=(dc_ == DC - 1),
                        )
                    t1 = fin.tile([P, D], F32, tag="t1")
                    nc.any.tensor_scalar_mul(t1[:], o_ps[:], recip[:, it, ii:ii + 1])
                    t2 = fin.tile([P, D], F32, tag="t2")
                    nc.vector.tensor_add(t2[:], t1[:], bvb[:])
                    nc.sync.dma_start(out_d[ic * P:(ic + 1) * P, :], t2[:])

            # ---- prologue ----
            KTR_LEAD, V_LEAD = 2, 4
            emit_weight_dmas()
            qTs0 = emit_q_tr(0)
            emit_keepalive()
            for jt in range(min(KTR_LEAD, NJ)):
                emit_ktr(jt)
                emit_keepalive()
            for jt in range(min(V_LEAD, NJ)):
                emit_vload(jt)
            emit_weight_prep()
            emit_a_proj(0, qTs0)

            # ---- main loop ----
            for it in range(NI):
                i0 = it * IT
                ctx_ps = [
                    acc_pool.tile([P, IT], F32, tag=f"ctx{dc_}", name=f"ctx_ps{dc_}")
                    for dc_ in range(DC)
                ]
                acc = accs_pool.tile([P, IT], F32R, tag="acc")

                def emit_ctx(psb, jt):
                    st, sp = (jt == 0), (jt == NJ - 1)
                    for dc_ in range(DC):
                        nc.tensor.matmul(
                            ctx_ps[dc_][:], vr[:, jt, dc_ * P:(dc_ + 1) * P], psb[:],
                            start=st, stop=sp,
                        )

                pending = []
                for jt in range(NJ):
                    if it == 0:
                        if jt + KTR_LEAD < NJ:
                            emit_ktr(jt + KTR_LEAD)
                        if jt + V_LEAD < NJ:
                            emit_vload(jt + V_LEAD)
                    if jt == (24 if it == 0 else 16) and it + 1 < NI:
                        emit_q_block(it + 1)
                    sps = s_pool.tile([P, IT], F32, tag="s")
                    for dc_ in range(DC):
                        nc.tensor.matmul(
                            sps[:], kT[:, dc_, jt * P:(jt + 1) * P], aT[:, dc_, i0:i0 + IT],
                            start=(dc_ == 0), stop=(dc_ == DC - 1),
                        )
                    psb = p_pool.tile([P, IT], F32R, tag="p")
                    nc.scalar.activation(psb[:], sps[:], EXP)
                    if jt == 0:
                        nc.vector.tensor_copy(acc[:], psb[:])
                    else:
                        nc.vector.tensor_add(acc[:], acc[:], psb[:])
                    pending.append((psb, jt))
                    if len(pending) > PIPE_DEPTH:
                        emit_ctx(*pending.pop(0))
                for args in pending:
                    emit_ctx(*args)

                # denominators, directly in column layout: den[i] = acc.T @ ones
                den_ps = o_pool.tile([P, ICPT], F32, tag="o")
                for ii in range(ICPT):
                    nc.tensor.matmul(
                        den_ps[:, ii:ii + 1],
                        acc[:, ii * P:(ii + 1) * P].bitcast(F32),
                        ones_f[:],
                        start=(ii == 0), stop=(ii == ICPT - 1),
                    )
                nc.any.tensor_copy(denpf[:, it, :], den_ps[:])

                if it >= 1:
                    emit_final(it - 1)
                for dc_ in range(DC):
                    nc.any.tensor_copy(ctxT[:, dc_, i0:i0 + IT], ctx_ps[dc_][:])
            emit_final(NI - 1)

    nc.compile()
    return nc


_CACHE: dict = {}


def _get_nc(S_q, S_k, num_devices):
    key = (S_q, S_k, num_devices)
    if key not in _CACHE:
        _CACHE[key] = build_attention(S_q, S_k, num_devices)
    return _CACHE[key]


def kernel(query, keys, values, Wq, bq, Wk, bk, Wv, bv, **_unused):
    """Full-input entry point: shards batch across 8 NeuronCores."""
    query = np.ascontiguousarray(query, dtype=np.float32)
    keys = np.ascontiguousarray(keys, dtype=np.float32)
    values = np.ascontiguousarray(values, dtype=np.float32)
    B, S_q, d = query.shape
    S_k = keys.shape[1]
    assert d == D and B == N_CORES

    nc = _get_nc(S_q, S_k, N_CORES)
    shared = {
        "Wq": np.ascontiguousarray(Wq, dtype=np.float32),
        "Wk": np.ascontiguousarray(Wk, dtype=np.float32),
        "Wv": np.ascontiguousarray(Wv, dtype=np.float32),
        "bq": np.ascontiguousarray(bq, dtype=np.float32),
        "bv": np.ascontiguousarray(bv, dtype=np.float32),
    }
    in_maps = [
        {"query": query[c], "keys": keys[c], "values": values[c], **shared}
        for c in range(B)
    ]
    res = run_bass_kernel_spmd(nc, in_maps, core_ids=list(range(N_CORES)))
    return np.stack([res.results[c]["out"] for c in range(B)], axis=0)


# revision 30
# speedup vs baseline: 1.0091x; 1.0040x over previous
"""Trainium2 Bass kernel for batched unscaled dot-product attention with
input projections (torch-Linear style):

    q = query @ Wq.T + bq ; k = keys @ Wk.T + bk ; v = values @ Wv.T + bv
    out = softmax(q @ k.T, axis=-1) @ v

Shapes: query/keys/values [B=8, S=4096, D=256]; W* [256, 256]; b* [256].

Strategy (data-parallel over batch, one batch element per NeuronCore):

Algebraic restructuring so NO tensor ever needs an HBM-side transpose and
the K/V projections fold away:
  - softmax(q@k.T) rows are invariant to adding per-row constants, so with
    A = query @ (Wq.T @ Wk) + 1*(bq @ Wk)   [4096, 256]
    softmax(q @ k.T) == softmax(A @ keys.T)   (bk drops out entirely).
  - out = P @ v = (P @ values) @ Wv.T + 1*bv  (P rows sum to 1), so the
    V projection is applied AFTER the attention-weighted sum.

On-chip pipeline per core (S^T layout — keys on PSUM partitions):
  1. prologue: ~3.4us of dummy matmuls open the PE clock-gate (HAM);
     Mw = Wq.T@Wk, c = Wk.T@bq on PE (tiny); PE-transpose query tiles;
     A^T = Mw^T q^T + c. keys-transposes and values-rounding are folded
     into the first i-tile's main loop for PE density.
  2. main loop over (i-tile of 512 query cols) x (j-chunk of 128 keys):
       S^T[j, i]  = kT.T @ A^T   (2 fp32r matmuls, PSUM)
       P^T        = exp(S^T)     (ScalarE, PSUM->SBUF; |scores| <~ 40 so
                                  exp() needs no max-subtraction in fp32)
       ctx^T     += values^T @ P^T  (2 fp32r matmuls, PSUM accum over j)
       acc       += P^T             (VectorE running sum for denominators)
     The NEXT i-tile's query transposes + A-projection are emitted
     mid-loop (jt==16) so their copies don't collide with the exp drain
     at the i-tile boundary.
  3. per i-tile: den[i] = acc.T @ ones — 4 tiny matmuls that land the
     denominators directly in [query-on-partition] column layout;
     recip on VectorE.
  4. out[i, d] = (ctx^T.T @ Wv^T) * recip + bv  (2 matmuls + DVE per
     128-row chunk), emitted one i-tile behind the main loop.

All big matmuls use float32r (full PE speed at free-dim>=256, ~1.5e-4
relative error vs 2.3e-3 for bf16 — measured on HW).
"""

import numpy as np

import concourse.bass as bass
import concourse.tile as tile
from concourse import bacc, mybir
from concourse.bass_utils import run_bass_kernel_spmd
from concourse.masks import make_identity

P = 128
D = 256
DC = D // P  # 2 chunks of the feature dim
IT = 512     # i-tile (query positions per main-loop tile)
ICPT = IT // P  # output row chunks per i-tile
N_CORES = 8
PIPE_DEPTH = 3  # ctx-matmul emission lag behind exp, in j-chunks

F32 = mybir.dt.float32
F32R = mybir.dt.float32r
EXP = mybir.ActivationFunctionType.Exp
BF16 = mybir.dt.bfloat16


def build_attention(S_q: int, S_k: int, num_devices: int = N_CORES):
    assert S_q % IT == 0 and S_k % P == 0
    NI = S_q // IT   # i-tiles
    NJ = S_k // P    # j-chunks
    NIC = S_q // P   # output row chunks

    nc = bacc.Bacc(
        "TRN2",
        target_bir_lowering=False,
        debug=False,
        enable_asserts=False,
        num_devices=num_devices,
    )

    q_d = nc.dram_tensor("query", [S_q, D], F32, kind="ExternalInput").ap()
    k_d = nc.dram_tensor("keys", [S_k, D], F32, kind="ExternalInput").ap()
    v_d = nc.dram_tensor("values", [S_k, D], F32, kind="ExternalInput").ap()
    wq_d = nc.dram_tensor("Wq", [D, D], F32, kind="ExternalInput").ap()
    wk_d = nc.dram_tensor("Wk", [D, D], F32, kind="ExternalInput").ap()
    wv_d = nc.dram_tensor("Wv", [D, D], F32, kind="ExternalInput").ap()
    bq_d = nc.dram_tensor("bq", [D], F32, kind="ExternalInput").ap()
    bv_d = nc.dram_tensor("bv", [D], F32, kind="ExternalInput").ap()
    out_d = nc.dram_tensor("out", [S_q, D], F32, kind="ExternalOutput").ap()

    with tile.TileContext(nc) as tc:
        with (
            tc.tile_pool(name="persist", bufs=1) as persist,
            tc.tile_pool(name="pre_in", bufs=6) as kin,
            tc.tile_pool(name="qts", bufs=2) as qts_pool,
            tc.tile_pool(name="s_ps", bufs=4, space="PSUM") as s_pool,
            tc.tile_pool(name="acc_ps", bufs=1, space="PSUM") as acc_pool,
            tc.tile_pool(name="o_ps", bufs=2, space="PSUM") as o_pool,
            tc.tile_pool(name="p_sb", bufs=PIPE_DEPTH + 2) as p_pool,
            tc.tile_pool(name="acc_sb", bufs=2) as accs_pool,
            tc.tile_pool(name="fin", bufs=3) as fin,
        ):
            kT = persist.tile([P, DC, S_k], F32R, tag="kT")       # keys^T
            aT = persist.tile([P, DC, S_q], F32R, tag="aT")       # A^T
            vr = persist.tile([P, NJ, D], BF16, tag="vr")         # values (bf16)
            ctxT = persist.tile([P, DC, S_q], F32R, tag="ctxT")   # (P@values)^T
            mw = persist.tile([P, DC, D], F32R, tag="mw")         # Wq.T@Wk
            wq = persist.tile([P, DC, D], F32, tag="wq")
            wk = persist.tile([P, DC, D], F32, tag="wk")
            wv = persist.tile([P, DC, D], F32, tag="wv")
            wvT = persist.tile([P, DC, D], F32R, tag="wvT")       # Wv^T
            cvec = persist.tile([P, DC], F32, tag="cvec")         # Wk.T@bq
            bqc = persist.tile([P, DC], F32, tag="bqc")
            wq_r = persist.tile([P, DC, D], F32R, tag="wq_r")
            wk_r = persist.tile([P, DC, D], F32R, tag="wk_r")
            bqc_r = persist.tile([P, DC], F32R, tag="bqc_r")
            ones_f = persist.tile([P, 1], F32, tag="ones_f")
            ones = persist.tile([P, 1], F32R, tag="ones")
            ident = persist.tile([P, P], F32, tag="ident")
            bvb = persist.tile([P, D], F32, tag="bvb")            # bv bcast
            denpf = persist.tile([P, NI, ICPT], F32, tag="denpf")
            recip = persist.tile([P, NI, ICPT], F32, tag="recip")
            wtile = persist.tile([P, IT], F32, tag="warm")

            nc.vector.memset(ones_f[:], 1.0)
            nc.vector.tensor_copy(ones[:], ones_f[:])
            make_identity(nc, ident[:])

            # HAM warmup: ~3.4us of real matmul activity un-throttles the PE
            # clock (1.2 -> 2.4 GHz) before the real pipeline begins.
            nc.vector.memset(wtile[:], 0.0)
            for _ in range(2):
                wps = s_pool.tile([P, IT], F32, tag="s")
                nc.tensor.matmul(wps[:], wtile[:, :P], wtile[:], start=True, stop=True)

            def emit_keepalive(n=P):
                # transposes don't register as PE-busy in the clock-gate's
                # activity window; a short real matmul does.
                wps = s_pool.tile([P, IT], F32, tag="s")
                nc.tensor.matmul(wps[:, :n], wtile[:, :P], wtile[:, :n], start=True, stop=True)

            def emit_weight_dmas():
                nc.gpsimd.dma_start(wq[:], wq_d.rearrange("(no ni) d -> ni no d", ni=P))
                nc.gpsimd.dma_start(wk[:], wk_d.rearrange("(no ni) d -> ni no d", ni=P))
                nc.gpsimd.dma_start(wv[:], wv_d.rearrange("(do p) di -> p do di", p=P))
                nc.gpsimd.dma_start(bqc[:], bq_d.rearrange("(no ni) -> ni no", ni=P))
                nc.gpsimd.dma_start(bvb[:], bv_d.unsqueeze(0).to_broadcast([P, D]))

            def emit_weight_prep():
                for dc_ in range(DC):
                    nc.any.tensor_copy(wq_r[:, dc_, :], wq[:, dc_, :])
                    nc.any.tensor_copy(wk_r[:, dc_, :], wk[:, dc_, :])
                nc.any.tensor_copy(bqc_r[:], bqc[:])
                for dic in range(DC):
                    mps = s_pool.tile([P, D], F32, tag="s")
                    for no in range(DC):
                        nc.tensor.matmul(
                            mps[:, :D],
                            wq_r[:, no, dic * P:(dic + 1) * P], wk_r[:, no, :],
                            start=(no == 0), stop=(no == DC - 1),
                        )
                    nc.any.tensor_copy(mw[:, dic, :], mps[:, :D])
                for kc in range(DC):
                    cps = s_pool.tile([P, 1], F32, tag="s")
                    for no in range(DC):
                        nc.tensor.matmul(
                            cps[:], wk[:, no, kc * P:(kc + 1) * P], bqc[:, no:no + 1],
                            start=(no == 0), stop=(no == DC - 1),
                        )
                    nc.any.tensor_copy(cvec[:, kc:kc + 1], cps[:])
                for a_ in range(DC):
                    for b_ in range(DC):
                        tps = s_pool.tile([P, P], F32, tag="s")
                        nc.tensor.transpose(tps[:, :P], wv[:, a_, b_ * P:(b_ + 1) * P], ident[:])
                        nc.any.tensor_copy(wvT[:, b_, a_ * P:(a_ + 1) * P], tps[:, :P])

            # ---- helpers ----
            def emit_ktr(jt):
                """DMA a 128-row chunk of keys, PE-transpose to kT."""
                ktile = kin.tile([P, D], F32, tag="ktile")
                nc.sync.dma_start(ktile[:], k_d[jt * P:(jt + 1) * P, :])
                for dc_ in range(DC):
                    tp = s_pool.tile([P, P], F32, tag="s")
                    nc.tensor.transpose(tp[:, :P], ktile[:, dc_ * P:(dc_ + 1) * P], ident[:])
                    nc.any.tensor_copy(kT[:, dc_, jt * P:(jt + 1) * P], tp[:, :P])

            def emit_vload(jt):
                vtile = kin.tile([P, D], F32, tag="vtile")
                nc.sync.dma_start(vtile[:], v_d[jt * P:(jt + 1) * P, :])
                nc.gpsimd.tensor_copy(vr[:, jt, :], vtile[:])

            def emit_q_tr(it):
                """Transpose 4 query chunks into a qTs staging tile."""
                qTs = qts_pool.tile([P, DC, IT], F32R, tag="qTs")
                for ii in range(ICPT):
                    r0 = it * IT + ii * P
                    qtile = kin.tile([P, D], F32, tag="qtile")
                    nc.sync.dma_start(qtile[:], q_d[r0:r0 + P, :])
                    for dc_ in range(DC):
                        tp = s_pool.tile([P, P], F32, tag="s")
                        nc.tensor.transpose(tp[:, :P], qtile[:, dc_ * P:(dc_ + 1) * P], ident[:])
                        nc.any.tensor_copy(qTs[:, dc_, ii * P:(ii + 1) * P], tp[:, :P])
                return qTs

            def emit_a_proj(it, qTs):
                for kc in range(DC):
                    aps = s_pool.tile([P, IT], F32, tag="s")
                    for dc_ in range(DC):
                        nc.tensor.matmul(
                            aps[:], mw[:, dc_, kc * P:(kc + 1) * P], qTs[:, dc_, :],
                            start=(dc_ == 0), stop=(dc_ == DC - 1),
                        )
                    nc.any.tensor_scalar_add(
                        aT[:, kc, it * IT:(it + 1) * IT], aps[:], cvec[:, kc:kc + 1]
                    )

            def emit_q_block(it):
                emit_a_proj(it, emit_q_tr(it))

            def emit_final(it):
                """recip + output projection + bias + store for one i-tile."""
                nc.vector.reciprocal(recip[:, it, :], denpf[:, it, :])
                for ii in range(ICPT):
                    ic = it * ICPT + ii
                    o_ps = o_pool.tile([P, D], F32, tag="o")
                    for dc_ in range(DC):
                        nc.tensor.matmul(
                            o_ps[:], ctxT[:, dc_, ic * P:(ic + 1) * P], wvT[:, dc_, :],
                            start=(dc_ == 0), stop=(dc_ == DC - 1),
                        )
                    t1 = fin.tile([P, D], F32, tag="t1")
                    nc.any.tensor_scalar_mul(t1[:], o_ps[:], recip[:, it, ii:ii + 1])
                    t2 = fin.tile([P, D], F32, tag="t2")
                    nc.vector.tensor_add(t2[:], t1[:], bvb[:])
                    nc.sync.dma_start(out_d[ic * P:(ic + 1) * P, :], t2[:])

            # ---- prologue ----
            KTR_LEAD, V_LEAD = 2, 4
            emit_weight_dmas()
            qTs0 = emit_q_tr(0)
            emit_keepalive()
            for jt in range(min(KTR_LEAD, NJ)):
                emit_ktr(jt)
                emit_keepalive()
            for jt in range(min(V_LEAD, NJ)):
                emit_vload(jt)
            emit_weight_prep()
            emit_a_proj(0, qTs0)

            # ---- main loop ----
            for it in range(NI):
                i0 = it * IT
                ctx_ps = [
                    acc_pool.tile([P, IT], F32, tag=f"ctx{dc_}", name=f"ctx_ps{dc_}")
                    for dc_ in range(DC)
                ]
                acc = accs_pool.tile([P, IT], F32, tag="acc")

                def emit_ctx(psb, jt):
                    st, sp = (jt == 0), (jt == NJ - 1)
                    for dc_ in range(DC):
                        nc.tensor.matmul(
                            ctx_ps[dc_][:], vr[:, jt, dc_ * P:(dc_ + 1) * P], psb[:],
                            start=st, stop=sp,
                        )

                pending = []
                for jt in range(NJ):
                    if it == 0:
                        if jt + KTR_LEAD < NJ:
                            emit_ktr(jt + KTR_LEAD)
                        if jt + V_LEAD < NJ:
                            emit_vload(jt + V_LEAD)
                    if jt == (24 if it == 0 else 16) and it + 1 < NI:
                        emit_q_block(it + 1)
                    sps = s_pool.tile([P, IT], F32, tag="s")
                    for dc_ in range(DC):
                        nc.tensor.matmul(
                            sps[:], kT[:, dc_, jt * P:(jt + 1) * P], aT[:, dc_, i0:i0 + IT],
                            start=(dc_ == 0), stop=(dc_ == DC - 1),
                        )
                    psb = p_pool.tile([P, IT], BF16, tag="p")
                    nc.scalar.activation(psb[:], sps[:], EXP)
                    if jt == 0:
                        nc.vector.tensor_copy(acc[:], psb[:])
                    else:
                        nc.vector.tensor_add(acc[:], acc[:], psb[:])
                    pending.append((psb, jt))
                    if len(pending) > PIPE_DEPTH:
                        emit_ctx(*pending.pop(0))
                for args in pending:
                    emit_ctx(*args)

                # denominators, directly in column layout: den[i] = acc.T @ ones
                den_ps = o_pool.tile([P, ICPT], F32, tag="o")
                for ii in range(ICPT):
                    nc.tensor.matmul(
                        den_ps[:, ii:ii + 1],
                        acc[:, ii * P:(ii + 1) * P],
                        ones_f[:],
                        start=(ii == 0), stop=(ii == ICPT - 1),
                    )
                nc.any.tensor_copy(denpf[:, it, :], den_ps[:])

                if it >= 1:
                    emit_final(it - 1)
                for dc_ in range(DC):
                    nc.any.tensor_copy(ctxT[:, dc_, i0:i0 + IT], ctx_ps[dc_][:])
            emit_final(NI - 1)

    nc.compile()
    return nc


_CACHE: dict = {}


def _get_nc(S_q, S_k, num_devices):
    key = (S_q, S_k, num_devices)
    if key not in _CACHE:
        _CACHE[key] = build_attention(S_q, S_k, num_devices)
    return _CACHE[key]


def kernel(query, keys, values, Wq, bq, Wk, bk, Wv, bv, **_unused):
    """Full-input entry point: shards batch across 8 NeuronCores."""
    query = np.ascontiguousarray(query, dtype=np.float32)
    keys = np.ascontiguousarray(keys, dtype=np.float32)
    values = np.ascontiguousarray(values, dtype=np.float32)
    B, S_q, d = query.shape
    S_k = keys.shape[1]
    assert d == D and B == N_CORES

    nc = _get_nc(S_q, S_k, N_CORES)
    shared = {
        "Wq": np.ascontiguousarray(Wq, dtype=np.float32),
        "Wk": np.ascontiguousarray(Wk, dtype=np.float32),
        "Wv": np.ascontiguousarray(Wv, dtype=np.float32),
        "bq": np.ascontiguousarray(bq, dtype=np.float32),
        "bv": np.ascontiguousarray(bv, dtype=np.float32),
    }
    in_maps = [
        {"query": query[c], "keys": keys[c], "values": values[c], **shared}
        for c in range(B)
    ]
    res = run_bass_kernel_spmd(nc, in_maps, core_ids=list(range(N_CORES)))
    return np.stack([res.results[c]["out"] for c in range(B)], axis=0)


# revision 31
# speedup vs baseline: 1.0159x; 1.0068x over previous
"""Trainium2 Bass kernel for batched unscaled dot-product attention with
input projections (torch-Linear style):

    q = query @ Wq.T + bq ; k = keys @ Wk.T + bk ; v = values @ Wv.T + bv
    out = softmax(q @ k.T, axis=-1) @ v

Shapes: query/keys/values [B=8, S=4096, D=256]; W* [256, 256]; b* [256].

Strategy (data-parallel over batch, one batch element per NeuronCore):

Algebraic restructuring so NO tensor ever needs an HBM-side transpose and
the K/V projections fold away:
  - softmax(q@k.T) rows are invariant to adding per-row constants, so with
    A = query @ (Wq.T @ Wk) + 1*(bq @ Wk)   [4096, 256]
    softmax(q @ k.T) == softmax(A @ keys.T)   (bk drops out entirely).
  - out = P @ v = (P @ values) @ Wv.T + 1*bv  (P rows sum to 1), so the
    V projection is applied AFTER the attention-weighted sum.

On-chip pipeline per core (S^T layout — keys on PSUM partitions):
  1. prologue: ~3.4us of dummy matmuls open the PE clock-gate (HAM);
     Mw = Wq.T@Wk, c = Wk.T@bq on PE (tiny); PE-transpose query tiles;
     A^T = Mw^T q^T + c. keys-transposes and values-rounding are folded
     into the first i-tile's main loop for PE density.
  2. main loop over (i-tile of 512 query cols) x (j-chunk of 128 keys):
       S^T[j, i]  = kT.T @ A^T   (2 fp32r matmuls, PSUM)
       P^T        = exp(S^T)     (ScalarE, PSUM->SBUF; |scores| <~ 40 so
                                  exp() needs no max-subtraction in fp32)
       ctx^T     += values^T @ P^T  (2 fp32r matmuls, PSUM accum over j)
       acc       += P^T             (VectorE running sum for denominators)
     The NEXT i-tile's query transposes + A-projection are emitted
     mid-loop (jt==16) so their copies don't collide with the exp drain
     at the i-tile boundary.
  3. per i-tile: den[i] = acc.T @ ones — 4 tiny matmuls that land the
     denominators directly in [query-on-partition] column layout;
     recip on VectorE.
  4. out[i, d] = (ctx^T.T @ Wv^T) * recip + bv  (2 matmuls + DVE per
     128-row chunk), emitted one i-tile behind the main loop.

All big matmuls use float32r (full PE speed at free-dim>=256, ~1.5e-4
relative error vs 2.3e-3 for bf16 — measured on HW).
"""

import numpy as np

import concourse.bass as bass
import concourse.tile as tile
from concourse import bacc, mybir
from concourse.bass_utils import run_bass_kernel_spmd
from concourse.masks import make_identity

P = 128
D = 256
DC = D // P  # 2 chunks of the feature dim
IT = 512     # i-tile (query positions per main-loop tile)
ICPT = IT // P  # output row chunks per i-tile
N_CORES = 8
PIPE_DEPTH = 3  # ctx-matmul emission lag behind exp, in j-chunks

F32 = mybir.dt.float32
F32R = mybir.dt.float32r
EXP = mybir.ActivationFunctionType.Exp
BF16 = mybir.dt.bfloat16


def build_attention(S_q: int, S_k: int, num_devices: int = N_CORES):
    assert S_q % IT == 0 and S_k % P == 0
    NI = S_q // IT   # i-tiles
    NJ = S_k // P    # j-chunks
    NIC = S_q // P   # output row chunks

    nc = bacc.Bacc(
        "TRN2",
        target_bir_lowering=False,
        debug=False,
        enable_asserts=False,
        num_devices=num_devices,
    )

    q_d = nc.dram_tensor("query", [S_q, D], F32, kind="ExternalInput").ap()
    k_d = nc.dram_tensor("keys", [S_k, D], F32, kind="ExternalInput").ap()
    v_d = nc.dram_tensor("values", [S_k, D], F32, kind="ExternalInput").ap()
    wq_d = nc.dram_tensor("Wq", [D, D], F32, kind="ExternalInput").ap()
    wk_d = nc.dram_tensor("Wk", [D, D], F32, kind="ExternalInput").ap()
    wv_d = nc.dram_tensor("Wv", [D, D], F32, kind="ExternalInput").ap()
    bq_d = nc.dram_tensor("bq", [D], F32, kind="ExternalInput").ap()
    bv_d = nc.dram_tensor("bv", [D], F32, kind="ExternalInput").ap()
    out_d = nc.dram_tensor("out", [S_q, D], F32, kind="ExternalOutput").ap()

    with tile.TileContext(nc) as tc:
        with (
            tc.tile_pool(name="persist", bufs=1) as persist,
            tc.tile_pool(name="pre_in", bufs=6) as kin,
            tc.tile_pool(name="qts", bufs=2) as qts_pool,
            tc.tile_pool(name="s_ps", bufs=4, space="PSUM") as s_pool,
            tc.tile_pool(name="acc_ps", bufs=1, space="PSUM") as acc_pool,
            tc.tile_pool(name="o_ps", bufs=2, space="PSUM") as o_pool,
            tc.tile_pool(name="p_sb", bufs=PIPE_DEPTH + 2) as p_pool,
            tc.tile_pool(name="acc_sb", bufs=2) as accs_pool,
            tc.tile_pool(name="fin", bufs=3) as fin,
        ):
            kT = persist.tile([P, DC, S_k], F32R, tag="kT")       # keys^T
            aT = persist.tile([P, DC, S_q], F32R, tag="aT")       # A^T
            vr = persist.tile([P, NJ, D], BF16, tag="vr")         # values (bf16)
            ctxT = persist.tile([P, DC, S_q], F32R, tag="ctxT")   # (P@values)^T
            mw = persist.tile([P, DC, D], F32R, tag="mw")         # Wq.T@Wk
            wq = persist.tile([P, DC, D], F32, tag="wq")
            wk = persist.tile([P, DC, D], F32, tag="wk")
            wv = persist.tile([P, DC, D], F32, tag="wv")
            wvT = persist.tile([P, DC, D], F32R, tag="wvT")       # Wv^T
            cvec = persist.tile([P, DC], F32, tag="cvec")         # Wk.T@bq
            bqc = persist.tile([P, DC], F32, tag="bqc")
            wq_r = persist.tile([P, DC, D], F32R, tag="wq_r")
            wk_r = persist.tile([P, DC, D], F32R, tag="wk_r")
            bqc_r = persist.tile([P, DC], F32R, tag="bqc_r")
            ones_f = persist.tile([P, 1], F32, tag="ones_f")
            ones = persist.tile([P, 1], F32R, tag="ones")
            ident = persist.tile([P, P], F32, tag="ident")
            bvb = persist.tile([P, D], F32, tag="bvb")            # bv bcast
            denpf = persist.tile([P, NI, ICPT], F32, tag="denpf")
            recip = persist.tile([P, NI, ICPT], F32, tag="recip")
            wtile = persist.tile([P, IT], F32, tag="warm")

            nc.vector.memset(ones_f[:], 1.0)
            nc.vector.tensor_copy(ones[:], ones_f[:])
            make_identity(nc, ident[:])

            # HAM warmup: ~3.4us of real matmul activity un-throttles the PE
            # clock (1.2 -> 2.4 GHz) before the real pipeline begins.
            nc.vector.memset(wtile[:], 0.0)
            for _ in range(2):
                wps = s_pool.tile([P, IT], F32, tag="s")
                nc.tensor.matmul(wps[:], wtile[:, :P], wtile[:], start=True, stop=True)

            def emit_keepalive(n=P):
                # transposes don't register as PE-busy in the clock-gate's
                # activity window; a short real matmul does.
                wps = s_pool.tile([P, IT], F32, tag="s")
                nc.tensor.matmul(wps[:, :n], wtile[:, :P], wtile[:, :n], start=True, stop=True)

            def emit_weight_dmas():
                nc.gpsimd.dma_start(wq[:], wq_d.rearrange("(no ni) d -> ni no d", ni=P))
                nc.gpsimd.dma_start(wk[:], wk_d.rearrange("(no ni) d -> ni no d", ni=P))
                nc.gpsimd.dma_start(wv[:], wv_d.rearrange("(do p) di -> p do di", p=P))
                nc.gpsimd.dma_start(bqc[:], bq_d.rearrange("(no ni) -> ni no", ni=P))
                nc.gpsimd.dma_start(bvb[:], bv_d.unsqueeze(0).to_broadcast([P, D]))

            def emit_weight_prep():
                for dc_ in range(DC):
                    nc.any.tensor_copy(wq_r[:, dc_, :], wq[:, dc_, :])
                    nc.any.tensor_copy(wk_r[:, dc_, :], wk[:, dc_, :])
                nc.any.tensor_copy(bqc_r[:], bqc[:])
                for dic in range(DC):
                    mps = s_pool.tile([P, D], F32, tag="s")
                    for no in range(DC):
                        nc.tensor.matmul(
                            mps[:, :D],
                            wq_r[:, no, dic * P:(dic + 1) * P], wk_r[:, no, :],
                            start=(no == 0), stop=(no == DC - 1),
                        )
                    nc.any.tensor_copy(mw[:, dic, :], mps[:, :D])
                for kc in range(DC):
                    cps = s_pool.tile([P, 1], F32, tag="s")
                    for no in range(DC):
                        nc.tensor.matmul(
                            cps[:], wk[:, no, kc * P:(kc + 1) * P], bqc[:, no:no + 1],
                            start=(no == 0), stop=(no == DC - 1),
                        )
                    nc.any.tensor_copy(cvec[:, kc:kc + 1], cps[:])
                for a_ in range(DC):
                    for b_ in range(DC):
                        tps = s_pool.tile([P, P], F32, tag="s")
                        nc.tensor.transpose(tps[:, :P], wv[:, a_, b_ * P:(b_ + 1) * P], ident[:])
                        nc.any.tensor_copy(wvT[:, b_, a_ * P:(a_ + 1) * P], tps[:, :P])

            # ---- helpers ----
            def emit_ktr(jt):
                """DMA a 128-row chunk of keys, PE-transpose to kT."""
                ktile = kin.tile([P, D], F32, tag="ktile")
                nc.sync.dma_start(ktile[:], k_d[jt * P:(jt + 1) * P, :])
                for dc_ in range(DC):
                    tp = s_pool.tile([P, P], F32, tag="s")
                    nc.tensor.transpose(tp[:, :P], ktile[:, dc_ * P:(dc_ + 1) * P], ident[:])
                    nc.any.tensor_copy(kT[:, dc_, jt * P:(jt + 1) * P], tp[:, :P])

            def emit_vload(jt):
                vtile = kin.tile([P, D], F32, tag="vtile")
                nc.sync.dma_start(vtile[:], v_d[jt * P:(jt + 1) * P, :])
                nc.gpsimd.tensor_copy(vr[:, jt, :], vtile[:])

            def emit_q_tr(it):
                """Transpose 4 query chunks into a qTs staging tile."""
                qTs = qts_pool.tile([P, DC, IT], F32R, tag="qTs")
                for ii in range(ICPT):
                    r0 = it * IT + ii * P
                    qtile = kin.tile([P, D], F32, tag="qtile")
                    nc.sync.dma_start(qtile[:], q_d[r0:r0 + P, :])
                    for dc_ in range(DC):
                        tp = s_pool.tile([P, P], F32, tag="s")
                        nc.tensor.transpose(tp[:, :P], qtile[:, dc_ * P:(dc_ + 1) * P], ident[:])
                        nc.any.tensor_copy(qTs[:, dc_, ii * P:(ii + 1) * P], tp[:, :P])
                return qTs

            def emit_a_proj(it, qTs):
                for kc in range(DC):
                    aps = s_pool.tile([P, IT], F32, tag="s")
                    for dc_ in range(DC):
                        nc.tensor.matmul(
                            aps[:], mw[:, dc_, kc * P:(kc + 1) * P], qTs[:, dc_, :],
                            start=(dc_ == 0), stop=(dc_ == DC - 1),
                        )
                    nc.any.tensor_scalar_add(
                        aT[:, kc, it * IT:(it + 1) * IT], aps[:], cvec[:, kc:kc + 1]
                    )

            def emit_q_block(it):
                emit_a_proj(it, emit_q_tr(it))

            def emit_final(it):
                """recip + output projection + bias + store for one i-tile."""
                nc.vector.reciprocal(recip[:, it, :], denpf[:, it, :])
                for ii in range(ICPT):
                    ic = it * ICPT + ii
                    o_ps = o_pool.tile([P, D], F32, tag="o")
                    for dc_ in range(DC):
                        nc.tensor.matmul(
                            o_ps[:], ctxT[:, dc_, ic * P:(ic + 1) * P], wvT[:, dc_, :],
                            start=(dc_ == 0), stop=(dc_ == DC - 1),
                        )
                    t1 = fin.tile([P, D], F32, tag="t1")
                    nc.any.tensor_scalar_mul(t1[:], o_ps[:], recip[:, it, ii:ii + 1])
                    t2 = fin.tile([P, D], F32, tag="t2")
                    nc.vector.tensor_add(t2[:], t1[:], bvb[:])
                    nc.sync.dma_start(out_d[ic * P:(ic + 1) * P, :], t2[:])

            # ---- prologue ----
            KTR_LEAD, V_LEAD = 2, 4
            emit_weight_dmas()
            qTs0 = emit_q_tr(0)
            emit_keepalive()
            for jt in range(min(KTR_LEAD, NJ)):
                emit_ktr(jt)
                emit_keepalive()
            for jt in range(min(V_LEAD, NJ)):
                emit_vload(jt)
            emit_weight_prep()
            emit_a_proj(0, qTs0)

            # ---- main loop ----
            for it in range(NI):
                i0 = it * IT
                ctx_ps = [
                    acc_pool.tile([P, IT], F32, tag=f"ctx{dc_}", name=f"ctx_ps{dc_}")
                    for dc_ in range(DC)
                ]
                acc = accs_pool.tile([P, IT], F32, tag="acc")

                def emit_ctx(psb, jt):
                    st, sp = (jt == 0), (jt == NJ - 1)
                    for dc_ in range(DC):
                        nc.tensor.matmul(
                            ctx_ps[dc_][:], vr[:, jt, dc_ * P:(dc_ + 1) * P], psb[:],
                            start=st, stop=sp,
                        )

                pending = []
                for jt in range(NJ):
                    if it == 0:
                        if jt + KTR_LEAD < NJ:
                            emit_ktr(jt + KTR_LEAD)
                        if jt + V_LEAD < NJ:
                            emit_vload(jt + V_LEAD)
                    if jt == (24 if it == 0 else 16) and it + 1 < NI:
                        emit_q_block(it + 1)
                    sps = s_pool.tile([P, IT], F32, tag="s")
                    for dc_ in range(DC):
                        nc.tensor.matmul(
                            sps[:], kT[:, dc_, jt * P:(jt + 1) * P], aT[:, dc_, i0:i0 + IT],
                            start=(dc_ == 0), stop=(dc_ == DC - 1),
                        )
                    psb = p_pool.tile([P, IT], BF16, tag="p")
                    nc.scalar.activation(psb[:], sps[:], EXP)
                    # denominator accumulation: bf16 pair-sums at 2x DVE rate,
                    # folded into the fp32 accumulator every other chunk.
                    if jt % 2 == 0:
                        prev_psb = psb
                    else:
                        pt = p_pool.tile([P, IT], BF16, tag="pt")
                        nc.vector.tensor_add(pt[:], prev_psb[:], psb[:])
                        if jt == 1:
                            nc.vector.tensor_copy(acc[:], pt[:])
                        else:
                            nc.vector.tensor_add(acc[:], acc[:], pt[:])
                    pending.append((psb, jt))
                    if len(pending) > PIPE_DEPTH:
                        emit_ctx(*pending.pop(0))
                for args in pending:
                    emit_ctx(*args)

                # denominators, directly in column layout: den[i] = acc.T @ ones
                den_ps = o_pool.tile([P, ICPT], F32, tag="o")
                for ii in range(ICPT):
                    nc.tensor.matmul(
                        den_ps[:, ii:ii + 1],
                        acc[:, ii * P:(ii + 1) * P],
                        ones_f[:],
                        start=(ii == 0), stop=(ii == ICPT - 1),
                    )
                nc.any.tensor_copy(denpf[:, it, :], den_ps[:])

                if it >= 1:
                    emit_final(it - 1)
                for dc_ in range(DC):
                    nc.any.tensor_copy(ctxT[:, dc_, i0:i0 + IT], ctx_ps[dc_][:])
            emit_final(NI - 1)

    nc.compile()
    return nc


_CACHE: dict = {}


def _get_nc(S_q, S_k, num_devices):
    key = (S_q, S_k, num_devices)
    if key not in _CACHE:
        _CACHE[key] = build_attention(S_q, S_k, num_devices)
    return _CACHE[key]


def kernel(query, keys, values, Wq, bq, Wk, bk, Wv, bv, **_unused):
    """Full-input entry point: shards batch across 8 NeuronCores."""
    query = np.ascontiguousarray(query, dtype=np.float32)
    keys = np.ascontiguousarray(keys, dtype=np.float32)
    values = np.ascontiguousarray(values, dtype=np.float32)
    B, S_q, d = query.shape
    S_k = keys.shape[1]
    assert d == D and B == N_CORES

    nc = _get_nc(S_q, S_k, N_CORES)
    shared = {
        "Wq": np.ascontiguousarray(Wq, dtype=np.float32),
        "Wk": np.ascontiguousarray(Wk, dtype=np.float32),
        "Wv": np.ascontiguousarray(Wv, dtype=np.float32),
        "bq": np.ascontiguousarray(bq, dtype=np.float32),
        "bv": np.ascontiguousarray(bv, dtype=np.float32),
    }
    in_maps = [
        {"query": query[c], "keys": keys[c], "values": values[c], **shared}
        for c in range(B)
    ]
    res = run_bass_kernel_spmd(nc, in_maps, core_ids=list(range(N_CORES)))
    return np.stack([res.results[c]["out"] for c in range(B)], axis=0)
